# revision 53
# baseline (speedup 1.0000x reference)
# Multi-headed self-attention (B=4, S=2048, D=1024, H=16) on 8 TRN2 NeuronCores.
#
# Sharding: tensor-parallel over heads. Core c computes heads 2c, 2c+1 (=128
# output columns) for all batches. Host pre-transposes x -> xT [D, B*S] (bf16)
# and the per-core weight slices -> [D, 128] (bf16). Device returns h^T plus
# the softmax denominator; host divides, transposes and concatenates.
#
# Per-core dataflow (bf16 matmul operands, fp32 PSUM):
#   1. Projections: QT/KT/VT [128(2 heads x 64), 8192] = W.T-slices @ xT,
#      accumulated over 8 d-chunks in PSUM; bias added during the PSUM->SBUF
#      copy (DVE per-partition scalar add, bf16 out).
#   2. V2 tiles [128 kpos, 64+1] per (b, head, kchunk) built by DMA XBAR
#      transpose of VT slices (2-byte dtype path, no PE/DVE cost); a constant
#      ones column per slot yields the softmax denominator through the PV
#      matmul (M=65). Each s-block projects V FIRST so the transposes hide
#      under the Q/K matmuls instead of stalling the first batch's PV chain.
#      XBAR transpose destinations must be 32-element aligned (corruption
#      otherwise) and sources work from any base partition.
#   3. Attention per (batch, qblock of 512): scoresT [128 kpos, 512 q] for
#      BOTH heads in one PE pass via row-tiling: head0 K=64 occupies PE row
#      strips 0-1 (tile_position (0,0)), head1 strips 2-3 ((64,0)) -- derived
#      automatically from the lhsT/rhs base partitions. exp on ScalarE with
#      fused 1/8 scale and per-partition mask bias (-10000*(1-m), exact
#      reference semantics; exp(-10000+x) == 0). PV accumulates h''^T [65,512]
#      over 16 kchunks per head. DVE copies PSUM->SBUF, DMA out.
#   4. No row-max subtraction (scores std ~0.4, exp safe); softmax is
#      shift-invariant. Host normalizes: h = num/den.
#
# Emission interleaves projection matmuls of batch b+1 between attention
# iterations of batch b so the PE stays dense while ScalarE exp (the fixed
# ~218us floor) streams.

import sys

import numpy as np

B, S, D, H = 4, 2048, 1024, 16
NC = 8
HPC = H // NC  # heads per core = 2
WH = D // H  # head width = 64
CW = HPC * WH  # per-core output width = 128
BS = B * S  # 8192
DCH = D // 128  # d chunks = 8
SB = BS // 512  # proj s-blocks = 16
QB = S // 512  # q blocks per batch = 4
KCH = S // 128  # k chunks per batch = 16
# v2 slot layout (192 cols): [VA(64) | onesA(1) | pad | VB@96(64) | onesB(1) | pad].
# 32-element offsets keep every dma_start_transpose destination 64B-aligned
# (misaligned XBAR transpose destinations silently corrupt).
V2W = 192
V2B = 96  # head B offset within slot

_CACHE = {}
DEBUG_DUMP = False


def _ensure_import():
    try:
        import concourse.bass  # noqa: F401
    except ImportError:
        sys.path.insert(0, "/opt/trn_rl_repo")
        import concourse.bass  # noqa: F401


def build_bass():
    if "nc" in _CACHE:
        return _CACHE["nc"]
    _ensure_import()
    import concourse.mybir as mybir
    import concourse.tile as tile
    from concourse import bacc

    f32 = mybir.dt.float32
    f32r = mybir.dt.float32r
    bf16 = mybir.dt.bfloat16
    AF = mybir.ActivationFunctionType

    nc = bacc.Bacc(
        "TRN2",
        target_bir_lowering=False,
        debug=False,
        enable_asserts=False,
        num_devices=NC,
    )
    xT_d = nc.dram_tensor("xT", (D, BS), bf16, kind="ExternalInput").ap()
    # weights host-prearranged to SBUF layout [p, 3*(c w)]; biases+maskbias
    # packed into one [128, 67] f32 tensor -- single DMA dispatch each.
    w_d = nc.dram_tensor("w_all", (128, 3 * DCH * CW), bf16, kind="ExternalInput").ap()
    cst_d = nc.dram_tensor("cst_all", (128, 3 + B * KCH), f32, kind="ExternalInput").ap()
    out_d = nc.dram_tensor("h_out", (130, BS), f32, kind="ExternalOutput").ap()
    if DEBUG_DUMP:
        qt_dump = nc.dram_tensor("qt_dump", (128, BS), mybir.dt.bfloat16, kind="ExternalOutput").ap()
        kt_dump = nc.dram_tensor("kt_dump", (128, BS), mybir.dt.bfloat16, kind="ExternalOutput").ap()
        vt_dump = nc.dram_tensor("vt_dump", (128, BS), mybir.dt.bfloat16, kind="ExternalOutput").ap()
        v2_dump = nc.dram_tensor("v2_dump", (B * 128, KCH * V2W), mybir.dt.bfloat16, kind="ExternalOutput").ap()

    with tile.TileContext(nc) as tc:
        with (
            tc.tile_pool(name="cst", bufs=1) as cst_pool,
            tc.tile_pool(name="qkv", bufs=1) as qkv_pool,
            tc.tile_pool(name="xt", bufs=4) as xt_pool,
            tc.tile_pool(name="pb", bufs=6) as pb_pool,
            tc.tile_pool(name="hts", bufs=4) as hts_pool,
            tc.tile_pool(name="ps", bufs=3, space="PSUM") as ps_pool,
            tc.tile_pool(name="ph", bufs=1, space="PSUM") as ph_pool,
        ):
            w_all = cst_pool.tile([128, 3 * DCH * CW], bf16, tag="w", name="w_all")
            nc.sync.dma_start(out=w_all, in_=w_d)
            wsbs = [w_all[:, i * DCH * CW : (i + 1) * DCH * CW] for i in range(3)]
            cst_all = cst_pool.tile([128, 3 + B * KCH], f32, tag="cst", name="cst_all")
            nc.sync.dma_start(out=cst_all, in_=cst_d)
            bsbs = [cst_all[:, i : i + 1] for i in range(3)]
            mb_sb = cst_all[:, 3:]

            qt = qkv_pool.tile([128, BS], bf16, tag="qt", name="qt")
            kt = qkv_pool.tile([128, BS], bf16, tag="kt", name="kt")
            vt = qkv_pool.tile([128, BS], bf16, tag="vt", name="vt")
            qkv_sb = [qt, kt, vt]

            # v2 tiles: per batch, 16 slots of [VA(64)|1|VB(64)|1]; +32 tail pad
            # so the 128-wide (FWL-eligible) PV lhsT slices stay in bounds.
            v2s = []
            for b in range(B):
                v2 = cst_pool.tile(
                    [128, KCH * V2W + 32], bf16, tag=f"v2_{b}", name=f"v2_{b}"
                )
                v2r = v2[:, 0 : KCH * V2W].rearrange("p (k c) -> p k c", c=V2W)
                nc.gpsimd.memset(v2r[:, :, WH], 1.0)
                nc.gpsimd.memset(v2r[:, :, V2B + WH], 1.0)
                v2s.append(v2)

            # ---- projection + v2-build generator: one yield per PE matmul ----
            def proj_work():
                for sb in range(SB):
                    b = sb // QB
                    xt_t = xt_pool.tile(
                        [128, DCH * 512], bf16, tag="xt", name=f"xt{sb}"
                    )
                    nc.sync.dma_start(
                        out=xt_t.rearrange("p (c w) -> p c w", c=DCH),
                        in_=xT_d[:, sb * 512 : (sb + 1) * 512].rearrange(
                            "(c p) w -> p c w", p=128
                        ),
                    )
                    xts = [xt_t[:, d * 512 : (d + 1) * 512] for d in range(DCH)]
                    # V first: its bias + XBAR transposes (v2 build) then
                    # overlap the Q/K matmul phase instead of stalling the
                    # just-in-time PV pipeline of the first batch.
                    tv = ps_pool.tile([128, 1024], f32, tag="big", name=f"v{sb}")
                    for d in range(DCH):
                        nc.tensor.matmul(
                            tv[:, 0:512],
                            wsbs[2][:, d * CW : (d + 1) * CW],
                            xts[d],
                            start=(d == 0),
                            stop=(d == DCH - 1),
                            skip_group_check=True,
                        )
                        yield
                    nc.vector.tensor_scalar_add(
                        vt[:, sb * 512 : (sb + 1) * 512], tv[:, 0:512], bsbs[2]
                    )
                    # v2 build per s-block: one batched XBAR transpose per
                    # head covering 4 slots (3D destination AP) -- per-call
                    # cost is overhead-dominated, so batching beats per-chunk
                    # calls ~4-16x on the sync queue; per-sblock granularity
                    # lets attention(b) start before the whole batch projects.
                    k0 = (sb % QB) * 4
                    v2r = v2s[b][:, 0 : KCH * V2W].rearrange(
                        "p (k c) -> p k c", c=V2W
                    )
                    nc.sync.dma_start_transpose(
                        out=v2r[:, k0 : k0 + 4, 0:WH],
                        in_=vt[0:WH, sb * 512 : (sb + 1) * 512],
                    )
                    nc.sync.dma_start_transpose(
                        out=v2r[:, k0 : k0 + 4, V2B : V2B + WH],
                        in_=vt[WH:128, sb * 512 : (sb + 1) * 512],
                    )
                    tqk = ps_pool.tile([128, 1024], f32, tag="big", name=f"qk{sb}")
                    for pi, half in ((0, 0), (1, 512)):
                        for d in range(DCH):
                            nc.tensor.matmul(
                                tqk[:, half : half + 512],
                                wsbs[pi][:, d * CW : (d + 1) * CW],
                                xts[d],
                                start=(d == 0),
                                stop=(d == DCH - 1),
                                skip_group_check=True,
                            )
                            yield
                        nc.vector.tensor_scalar_add(
                            qkv_sb[pi][:, sb * 512 : (sb + 1) * 512],
                            tqk[:, half : half + 512],
                            bsbs[pi],
                        )

            gen = proj_work()
            pulled = [0]

            def pull(n):
                for _ in range(n):
                    try:
                        next(gen)
                        pulled[0] += 1
                    except StopIteration:
                        return

            def ensure_sblock(s):
                # emit projection work until s-block s is fully emitted --
                # dependency tracking follows emission order, so consumers
                # must always be emitted after their producers.
                while pulled[0] < 24 * (s + 1):
                    try:
                        next(gen)
                        pulled[0] += 1
                    except StopIteration:
                        return

            def emit_attention(b):
                base = b * S
                v2 = v2s[b]

                def emit_scores(qb, kc):
                    # scores pair + exp for one k-chunk (both heads)
                    ensure_sblock(4 * b + max(qb, kc // 4))
                    qs = base + qb * 512
                    sc = ps_pool.tile(
                        [128, 1024], f32, tag="big", name=f"sc{b}_{qb}_{kc}"
                    )
                    kcol = base + kc * 128
                    nc.tensor.matmul(
                        sc[:, 0:512],
                        kt[0:WH, kcol : kcol + 128],
                        qt[0:WH, qs : qs + 512],
                        start=True,
                        stop=True,
                        skip_group_check=True,
                    )
                    nc.tensor.matmul(
                        sc[:, 512:1024],
                        kt[WH:128, kcol : kcol + 128],
                        qt[WH:128, qs : qs + 512],
                        start=True,
                        stop=True,
                        skip_group_check=True,
                    )
                    pb = pb_pool.tile(
                        [128, 1024], bf16, tag="pb", name=f"pb{b}_{qb}_{kc}"
                    )
                    nc.scalar.activation(
                        pb,
                        sc,
                        AF.Exp,
                        bias=mb_sb[:, b * KCH + kc : b * KCH + kc + 1],
                        scale=0.125,
                    )
                    return pb

                for qb in range(QB):
                    qs = base + qb * 512
                    phA = ph_pool.tile([65, 512], f32, tag="pha", name=f"phA{b}_{qb}")
                    phB = ph_pool.tile([65, 512], f32, tag="phb", name=f"phB{b}_{qb}")
                    # 1-iteration software pipeline: scores(kc+1) is emitted
                    # before PV(kc), so the PE queue never head-blocks on the
                    # exp chain while scores work is available.
                    pb_next = emit_scores(qb, 0)
                    for kc in range(KCH):
                        pb = pb_next
                        if kc + 1 < KCH:
                            pull(
                                4
                                if (b, qb) == (0, 0)
                                else (2 if kc % 2 == 0 else 1)
                            )
                            pb_next = emit_scores(qb, kc + 1)
                        nc.tensor.matmul(
                            phA,
                            v2[:, kc * V2W : kc * V2W + 65],
                            pb[:, 0:512],
                            start=(kc == 0),
                            stop=(kc == KCH - 1),
                            skip_group_check=True,
                        )
                        nc.tensor.matmul(
                            phB,
                            v2[:, kc * V2W + V2B : kc * V2W + V2B + 65],
                            pb[:, 512:1024],
                            start=(kc == 0),
                            stop=(kc == KCH - 1),
                            skip_group_check=True,
                        )
                    htsA = hts_pool.tile([65, 512], f32, tag="hta", name=f"htsA{b}_{qb}")
                    htsB = hts_pool.tile([65, 512], f32, tag="htb", name=f"htsB{b}_{qb}")
                    nc.vector.tensor_copy(htsA, phA)
                    nc.vector.tensor_copy(htsB, phB)
                    nc.gpsimd.dma_start(
                        out=out_d[0:65, qs : qs + 512], in_=htsA
                    )
                    nc.gpsimd.dma_start(
                        out=out_d[65:130, qs : qs + 512], in_=htsB
                    )

            # prime: batch 0's full projection.
            pull(96)
            # All-engine barrier after batch 0's full projection: attention
            # instructions that consume just-in-time projection output race
            # it on fresh NEFF loads (observed ~50% NaN in the first q-block
            # -- one racy k-chunk poisons the whole PV accumulation). With
            # batch 0 fully projected and fenced, every later producer is
            # consumed a full batch (~60us) after emission.
            tc.strict_bb_all_engine_barrier()
            for b in range(B):
                emit_attention(b)
            pull(10000)  # drain any leftover projection work
            if DEBUG_DUMP:
                nc.sync.dma_start(out=qt_dump, in_=qt)
                nc.sync.dma_start(out=kt_dump, in_=kt)
                nc.sync.dma_start(out=vt_dump, in_=vt)
                for b in range(B):
                    nc.sync.dma_start(
                        out=v2_dump[b * 128 : (b + 1) * 128, :], in_=v2s[b]
                    )

    nc.compile()
    _CACHE["nc"] = nc
    return nc


def _warrange(W, cols, bf16):
    # [D, 128] slice -> SBUF layout [p(128), c(8)*w(128)]
    wT = np.asarray(W, np.float32)[cols, :].T  # (D, CW)
    return np.ascontiguousarray(
        wT.reshape(DCH, 128, CW).transpose(1, 0, 2).reshape(128, DCH * CW).astype(bf16)
    )


def make_in_maps(x, mask, Wq, bq, Wk, bk, Wv, bv):
    import ml_dtypes

    bf16 = ml_dtypes.bfloat16
    x = np.asarray(x, dtype=np.float32)
    xT16 = np.ascontiguousarray(x.reshape(BS, D).T.astype(bf16))
    mb = np.ascontiguousarray(
        (-10000.0 * (1.0 - np.asarray(mask, dtype=np.float32)))
        .reshape(B, KCH, 128)
        .transpose(2, 0, 1)
        .reshape(128, B * KCH)
    )
    in_maps = []
    for c in range(NC):
        cols = slice(c * CW, (c + 1) * CW)
        w_all = np.concatenate(
            [_warrange(W, cols, bf16) for W in (Wq, Wk, Wv)], axis=1
        )
        cst = np.concatenate(
            [
                np.asarray(bq, np.float32)[cols, None],
                np.asarray(bk, np.float32)[cols, None],
                np.asarray(bv, np.float32)[cols, None],
                mb,
            ],
            axis=1,
        ).astype(np.float32)
        in_maps.append(
            {
                "xT": xT16,
                "w_all": np.ascontiguousarray(w_all),
                "cst_all": np.ascontiguousarray(cst),
            }
        )
    return in_maps


def assemble(results):
    out = np.empty((BS, D), dtype=np.float32)
    for c in range(NC):
        raw = results[c]["h_out"]  # [130, BS] f32
        for j in range(HPC):
            num = raw[j * 65 : j * 65 + WH]  # [64, BS]
            den = raw[j * 65 + WH : j * 65 + WH + 1]  # [1, BS]
            hcol = (c * HPC + j) * WH
            out[:, hcol : hcol + WH] = (num / den).T
    return out.reshape(B, S, D)


def kernel(x, mask, Wq, bq, Wk, bk, Wv, bv, **run_kwargs):
    _ensure_import()
    from concourse.bass_utils import run_bass_kernel_spmd

    nc = build_bass()
    in_maps = make_in_maps(x, mask, Wq, bq, Wk, bk, Wv, bv)
    res = run_bass_kernel_spmd(nc, in_maps, core_ids=list(range(NC)), **run_kwargs)
    _CACHE["last_results"] = res
    return assemble(res.results)


# revision 54
# speedup vs baseline: 1.0020x; 1.0020x over previous
# Multi-headed self-attention (B=4, S=2048, D=1024, H=16) on 8 TRN2 NeuronCores.
#
# Sharding: tensor-parallel over heads. Core c computes heads 2c, 2c+1 (=128
# output columns) for all batches. Host pre-transposes x -> xT [D, B*S] (bf16)
# and the per-core weight slices -> [D, 128] (bf16). Device returns h^T plus
# the softmax denominator; host divides, transposes and concatenates.
#
# Per-core dataflow (bf16 matmul operands, fp32 PSUM):
#   1. Projections: QT/KT/VT [128(2 heads x 64), 8192] = W.T-slices @ xT,
#      accumulated over 8 d-chunks in PSUM; bias added during the PSUM->SBUF
#      copy (DVE per-partition scalar add, bf16 out).
#   2. V2 tiles [128 kpos, 64+1] per (b, head, kchunk) built by DMA XBAR
#      transpose of VT slices (2-byte dtype path, no PE/DVE cost); a constant
#      ones column per slot yields the softmax denominator through the PV
#      matmul (M=65). Each s-block projects V FIRST so the transposes hide
#      under the Q/K matmuls instead of stalling the first batch's PV chain.
#      XBAR transpose destinations must be 32-element aligned (corruption
#      otherwise) and sources work from any base partition.
#   3. Attention per (batch, qblock of 512): scoresT [128 kpos, 512 q] for
#      BOTH heads in one PE pass via row-tiling: head0 K=64 occupies PE row
#      strips 0-1 (tile_position (0,0)), head1 strips 2-3 ((64,0)) -- derived
#      automatically from the lhsT/rhs base partitions. exp on ScalarE with
#      fused 1/8 scale and per-partition mask bias (-10000*(1-m), exact
#      reference semantics; exp(-10000+x) == 0). PV accumulates h''^T [65,512]
#      over 16 kchunks per head. DVE copies PSUM->SBUF, DMA out.
#   4. No row-max subtraction (scores std ~0.4, exp safe); softmax is
#      shift-invariant. Host normalizes: h = num/den.
#
# Emission interleaves projection matmuls of batch b+1 between attention
# iterations of batch b so the PE stays dense while ScalarE exp (the fixed
# ~218us floor) streams.

import sys

import numpy as np

B, S, D, H = 4, 2048, 1024, 16
NC = 8
HPC = H // NC  # heads per core = 2
WH = D // H  # head width = 64
CW = HPC * WH  # per-core output width = 128
BS = B * S  # 8192
DCH = D // 128  # d chunks = 8
SB = BS // 512  # proj s-blocks = 16
QB = S // 512  # q blocks per batch = 4
KCH = S // 128  # k chunks per batch = 16
# v2 slot layout (192 cols): [VA(64) | onesA(1) | pad | VB@96(64) | onesB(1) | pad].
# 32-element offsets keep every dma_start_transpose destination 64B-aligned
# (misaligned XBAR transpose destinations silently corrupt).
V2W = 192
V2B = 96  # head B offset within slot

_CACHE = {}
DEBUG_DUMP = False


def _ensure_import():
    try:
        import concourse.bass  # noqa: F401
    except ImportError:
        sys.path.insert(0, "/opt/trn_rl_repo")
        import concourse.bass  # noqa: F401


def build_bass():
    if "nc" in _CACHE:
        return _CACHE["nc"]
    _ensure_import()
    import concourse.mybir as mybir
    import concourse.tile as tile
    from concourse import bacc

    f32 = mybir.dt.float32
    f32r = mybir.dt.float32r
    bf16 = mybir.dt.bfloat16
    AF = mybir.ActivationFunctionType

    nc = bacc.Bacc(
        "TRN2",
        target_bir_lowering=False,
        debug=False,
        enable_asserts=False,
        num_devices=NC,
    )
    xT_d = nc.dram_tensor("xT", (D, BS), bf16, kind="ExternalInput").ap()
    # weights host-prearranged to SBUF layout [p, 3*(c w)]; biases+maskbias
    # packed into one [128, 67] f32 tensor -- single DMA dispatch each.
    w_d = nc.dram_tensor("w_all", (128, 3 * DCH * CW), bf16, kind="ExternalInput").ap()
    cst_d = nc.dram_tensor("cst_all", (128, 3 + B * KCH), f32, kind="ExternalInput").ap()
    out_d = nc.dram_tensor("h_out", (130, BS), f32, kind="ExternalOutput").ap()
    if DEBUG_DUMP:
        qt_dump = nc.dram_tensor("qt_dump", (128, BS), mybir.dt.bfloat16, kind="ExternalOutput").ap()
        kt_dump = nc.dram_tensor("kt_dump", (128, BS), mybir.dt.bfloat16, kind="ExternalOutput").ap()
        vt_dump = nc.dram_tensor("vt_dump", (128, BS), mybir.dt.bfloat16, kind="ExternalOutput").ap()
        v2_dump = nc.dram_tensor("v2_dump", (B * 128, KCH * V2W), mybir.dt.bfloat16, kind="ExternalOutput").ap()

    with tile.TileContext(nc) as tc:
        with (
            tc.tile_pool(name="cst", bufs=1) as cst_pool,
            tc.tile_pool(name="qkv", bufs=1) as qkv_pool,
            tc.tile_pool(name="xt", bufs=3) as xt_pool,
            tc.tile_pool(name="pb", bufs=3) as pb_pool,
            tc.tile_pool(name="hts", bufs=2) as hts_pool,
            tc.tile_pool(name="ps", bufs=3, space="PSUM") as ps_pool,
            tc.tile_pool(name="ph", bufs=1, space="PSUM") as ph_pool,
        ):
            w_all = cst_pool.tile([128, 3 * DCH * CW], bf16, tag="w", name="w_all")
            nc.sync.dma_start(out=w_all, in_=w_d)
            wsbs = [w_all[:, i * DCH * CW : (i + 1) * DCH * CW] for i in range(3)]
            cst_all = cst_pool.tile([128, 3 + B * KCH], f32, tag="cst", name="cst_all")
            nc.sync.dma_start(out=cst_all, in_=cst_d)
            bsbs = [cst_all[:, i : i + 1] for i in range(3)]
            mb_sb = cst_all[:, 3:]

            qt = qkv_pool.tile([128, BS], bf16, tag="qt", name="qt")
            kt = qkv_pool.tile([128, BS], bf16, tag="kt", name="kt")
            vt = qkv_pool.tile([128, BS], bf16, tag="vt", name="vt")
            qkv_sb = [qt, kt, vt]

            # v2 tiles: per batch, 16 slots of [VA(64)|1|VB(64)|1]; +32 tail pad
            # so the 128-wide (FWL-eligible) PV lhsT slices stay in bounds.
            v2s = []
            for b in range(B):
                v2 = cst_pool.tile(
                    [128, KCH * V2W + 32], bf16, tag=f"v2_{b}", name=f"v2_{b}"
                )
                v2r = v2[:, 0 : KCH * V2W].rearrange("p (k c) -> p k c", c=V2W)
                nc.gpsimd.memset(v2r[:, :, WH], 1.0)
                nc.gpsimd.memset(v2r[:, :, V2B + WH], 1.0)
                v2s.append(v2)

            # ---- projection + v2-build generator: one yield per PE matmul ----
            def proj_work():
                for sb in range(SB):
                    b = sb // QB
                    xt_t = xt_pool.tile(
                        [128, DCH * 512], bf16, tag="xt", name=f"xt{sb}"
                    )
                    nc.sync.dma_start(
                        out=xt_t.rearrange("p (c w) -> p c w", c=DCH),
                        in_=xT_d[:, sb * 512 : (sb + 1) * 512].rearrange(
                            "(c p) w -> p c w", p=128
                        ),
                    )
                    xts = [xt_t[:, d * 512 : (d + 1) * 512] for d in range(DCH)]
                    # V first: its bias + XBAR transposes (v2 build) then
                    # overlap the Q/K matmul phase instead of stalling the
                    # just-in-time PV pipeline of the first batch.
                    tv = ps_pool.tile([128, 1024], f32, tag="big", name=f"v{sb}")
                    for d in range(DCH):
                        nc.tensor.matmul(
                            tv[:, 0:512],
                            wsbs[2][:, d * CW : (d + 1) * CW],
                            xts[d],
                            start=(d == 0),
                            stop=(d == DCH - 1),
                            skip_group_check=True,
                        )
                        yield
                    nc.vector.tensor_scalar_add(
                        vt[:, sb * 512 : (sb + 1) * 512], tv[:, 0:512], bsbs[2]
                    )
                    # v2 build per s-block: one batched XBAR transpose per
                    # head covering 4 slots (3D destination AP) -- per-call
                    # cost is overhead-dominated, so batching beats per-chunk
                    # calls ~4-16x on the sync queue; per-sblock granularity
                    # lets attention(b) start before the whole batch projects.
                    k0 = (sb % QB) * 4
                    v2r = v2s[b][:, 0 : KCH * V2W].rearrange(
                        "p (k c) -> p k c", c=V2W
                    )
                    nc.sync.dma_start_transpose(
                        out=v2r[:, k0 : k0 + 4, 0:WH],
                        in_=vt[0:WH, sb * 512 : (sb + 1) * 512],
                    )
                    nc.sync.dma_start_transpose(
                        out=v2r[:, k0 : k0 + 4, V2B : V2B + WH],
                        in_=vt[WH:128, sb * 512 : (sb + 1) * 512],
                    )
                    tqk = ps_pool.tile([128, 1024], f32, tag="big", name=f"qk{sb}")
                    for pi, half in ((0, 0), (1, 512)):
                        for d in range(DCH):
                            nc.tensor.matmul(
                                tqk[:, half : half + 512],
                                wsbs[pi][:, d * CW : (d + 1) * CW],
                                xts[d],
                                start=(d == 0),
                                stop=(d == DCH - 1),
                                skip_group_check=True,
                            )
                            yield
                        nc.vector.tensor_scalar_add(
                            qkv_sb[pi][:, sb * 512 : (sb + 1) * 512],
                            tqk[:, half : half + 512],
                            bsbs[pi],
                        )

            gen = proj_work()
            pulled = [0]

            def pull(n):
                for _ in range(n):
                    try:
                        next(gen)
                        pulled[0] += 1
                    except StopIteration:
                        return

            def ensure_sblock(s):
                # emit projection work until s-block s is fully emitted --
                # dependency tracking follows emission order, so consumers
                # must always be emitted after their producers.
                while pulled[0] < 24 * (s + 1):
                    try:
                        next(gen)
                        pulled[0] += 1
                    except StopIteration:
                        return

            def emit_attention(b):
                base = b * S
                v2 = v2s[b]

                def emit_scores(qb, kc):
                    # scores pair + exp for one k-chunk (both heads)
                    ensure_sblock(4 * b + max(qb, kc // 4))
                    qs = base + qb * 512
                    sc = ps_pool.tile(
                        [128, 1024], f32, tag="big", name=f"sc{b}_{qb}_{kc}"
                    )
                    kcol = base + kc * 128
                    nc.tensor.matmul(
                        sc[:, 0:512],
                        kt[0:WH, kcol : kcol + 128],
                        qt[0:WH, qs : qs + 512],
                        start=True,
                        stop=True,
                        skip_group_check=True,
                    )
                    nc.tensor.matmul(
                        sc[:, 512:1024],
                        kt[WH:128, kcol : kcol + 128],
                        qt[WH:128, qs : qs + 512],
                        start=True,
                        stop=True,
                        skip_group_check=True,
                    )
                    pb = pb_pool.tile(
                        [128, 1024], bf16, tag="pb", name=f"pb{b}_{qb}_{kc}"
                    )
                    nc.scalar.activation(
                        pb,
                        sc,
                        AF.Exp,
                        bias=mb_sb[:, b * KCH + kc : b * KCH + kc + 1],
                        scale=0.125,
                    )
                    return pb

                for qb in range(QB):
                    qs = base + qb * 512
                    phA = ph_pool.tile([65, 512], f32, tag="pha", name=f"phA{b}_{qb}")
                    phB = ph_pool.tile([65, 512], f32, tag="phb", name=f"phB{b}_{qb}")
                    # 1-iteration software pipeline: scores(kc+1) is emitted
                    # before PV(kc), so the PE queue never head-blocks on the
                    # exp chain while scores work is available.
                    pb_next = emit_scores(qb, 0)
                    for kc in range(KCH):
                        pb = pb_next
                        if kc + 1 < KCH:
                            pull(
                                4
                                if (b, qb) == (0, 0)
                                else (2 if kc % 2 == 0 else 1)
                            )
                            pb_next = emit_scores(qb, kc + 1)
                        nc.tensor.matmul(
                            phA,
                            v2[:, kc * V2W : kc * V2W + 65],
                            pb[:, 0:512],
                            start=(kc == 0),
                            stop=(kc == KCH - 1),
                            skip_group_check=True,
                        )
                        nc.tensor.matmul(
                            phB,
                            v2[:, kc * V2W + V2B : kc * V2W + V2B + 65],
                            pb[:, 512:1024],
                            start=(kc == 0),
                            stop=(kc == KCH - 1),
                            skip_group_check=True,
                        )
                    htsA = hts_pool.tile([65, 512], f32, tag="hta", name=f"htsA{b}_{qb}")
                    htsB = hts_pool.tile([65, 512], f32, tag="htb", name=f"htsB{b}_{qb}")
                    nc.vector.tensor_copy(htsA, phA)
                    nc.vector.tensor_copy(htsB, phB)
                    nc.gpsimd.dma_start(
                        out=out_d[0:65, qs : qs + 512], in_=htsA
                    )
                    nc.gpsimd.dma_start(
                        out=out_d[65:130, qs : qs + 512], in_=htsB
                    )

            # prime: batch 0's full projection.
            pull(96)
            # All-engine barrier after batch 0's full projection: attention
            # instructions that consume just-in-time projection output race
            # it on fresh NEFF loads (observed ~50% NaN in the first q-block
            # -- one racy k-chunk poisons the whole PV accumulation). With
            # batch 0 fully projected and fenced, every later producer is
            # consumed a full batch (~60us) after emission.
            tc.strict_bb_all_engine_barrier()
            for b in range(B):
                emit_attention(b)
            pull(10000)  # drain any leftover projection work
            if DEBUG_DUMP:
                nc.sync.dma_start(out=qt_dump, in_=qt)
                nc.sync.dma_start(out=kt_dump, in_=kt)
                nc.sync.dma_start(out=vt_dump, in_=vt)
                for b in range(B):
                    nc.sync.dma_start(
                        out=v2_dump[b * 128 : (b + 1) * 128, :], in_=v2s[b]
                    )

    nc.compile()
    _CACHE["nc"] = nc
    return nc


def _warrange(W, cols, bf16):
    # [D, 128] slice -> SBUF layout [p(128), c(8)*w(128)]
    wT = np.asarray(W, np.float32)[cols, :].T  # (D, CW)
    return np.ascontiguousarray(
        wT.reshape(DCH, 128, CW).transpose(1, 0, 2).reshape(128, DCH * CW).astype(bf16)
    )


def make_in_maps(x, mask, Wq, bq, Wk, bk, Wv, bv):
    import ml_dtypes

    bf16 = ml_dtypes.bfloat16
    x = np.asarray(x, dtype=np.float32)
    xT16 = np.ascontiguousarray(x.reshape(BS, D).T.astype(bf16))
    mb = np.ascontiguousarray(
        (-10000.0 * (1.0 - np.asarray(mask, dtype=np.float32)))
        .reshape(B, KCH, 128)
        .transpose(2, 0, 1)
        .reshape(128, B * KCH)
    )
    in_maps = []
    for c in range(NC):
        cols = slice(c * CW, (c + 1) * CW)
        w_all = np.concatenate(
            [_warrange(W, cols, bf16) for W in (Wq, Wk, Wv)], axis=1
        )
        cst = np.concatenate(
            [
                np.asarray(bq, np.float32)[cols, None],
                np.asarray(bk, np.float32)[cols, None],
                np.asarray(bv, np.float32)[cols, None],
                mb,
            ],
            axis=1,
        ).astype(np.float32)
        in_maps.append(
            {
                "xT": xT16,
                "w_all": np.ascontiguousarray(w_all),
                "cst_all": np.ascontiguousarray(cst),
            }
        )
    return in_maps


def assemble(results):
    out = np.empty((BS, D), dtype=np.float32)
    for c in range(NC):
        raw = results[c]["h_out"]  # [130, BS] f32
        for j in range(HPC):
            num = raw[j * 65 : j * 65 + WH]  # [64, BS]
            den = raw[j * 65 + WH : j * 65 + WH + 1]  # [1, BS]
            hcol = (c * HPC + j) * WH
            out[:, hcol : hcol + WH] = (num / den).T
    return out.reshape(B, S, D)


def kernel(x, mask, Wq, bq, Wk, bk, Wv, bv, **run_kwargs):
    _ensure_import()
    from concourse.bass_utils import run_bass_kernel_spmd

    nc = build_bass()
    in_maps = make_in_maps(x, mask, Wq, bq, Wk, bk, Wv, bv)
    res = run_bass_kernel_spmd(nc, in_maps, core_ids=list(range(NC)), **run_kwargs)
    _CACHE["last_results"] = res
    return assemble(res.results)
